# revision 1
# baseline (speedup 1.0000x reference)
"""Trainium2 Bass kernel for the 14-wire quantum autoencoder swap test.

Math reduction: reference wires 10-13 stay |0> until the swap test, so
P(aux=1) = (1 - q)/2 where q = sum_{i mod 8 == 0} |c_i|^2 over the 10-qubit
state c (wires 0-9) after AngleEmbedding + BasicEntanglerLayers.

Device layout (per core, 32 samples):
  state re/im tiles [128, 256] f32
  partition p = w9*64 + w8*32 + w7*16 + w6*8 + w5*4 + w4*2 + w3
  free      f = bh*128 + g*16 + bl   (b = bh*16+bl, g = w0*4 + w1*2 + w2)

The free axis splits into two independent half-batch streams (bh = 0/1) so
the DVE rotation phase of one half overlaps the PE matmul phase of the other.

Per entangler layer (gate order: RX all wires, then CNOT(w,w+1) w=0..9):
  - RX w0,w1 as tan-form scalar_tensor_tensor ops (cos deferred to the final
    affine), RX w2 fused with the pi = C12*C01 output permutation.
  - RX w3..w9 + C34..C89 as one host-built 128x128 complex matrix K2;
    C23 applied by using K2 on even-g columns and K2b = K2*X_w3 on odd-g
    columns (fp32 matmuls accumulating re/im in PSUM).
  - PSUM->SBUF copyback (ACT engine) folds C90: on w9=1 partitions g ^= 4.
Final: |.|^2 on partitions 0..15 (trash=000), per-sample reduce, ones-matmul
partition sum, affine 0.5 - 0.5*T^2*q.
"""
import numpy as np

NCORES = 8
B_CORE = 32
HB = 16            # half-batch
DEPTH = 4
NQ = 10

# packed const tile column layout
C_SCAL = 0         # [128p, 32]
C_SEED_RE = 32     # [32p, 32]
C_SEED_IM = 64
C_DBL_C = 96       # [32p, 2] (w8, w9)
C_DBL_S = 98
C_FIMN = 104       # [32p, 256]
C_FRE = 360
C_FIM = 616
C_TOT = 872

# ---------------------------------------------------------------------------
# Host-side plan construction
# ---------------------------------------------------------------------------


def _perm_matrix(perm):
    m = np.zeros((len(perm), len(perm)), dtype=np.float64)
    for src, dst in enumerate(perm):
        m[dst, src] = 1.0
    return m


def _cnot_chain_perm_p():
    perm = np.zeros(128, dtype=np.int64)
    for p in range(128):
        w = [(p >> k) & 1 for k in range(7)]
        for k in range(6):
            w[k + 1] ^= w[k]
        perm[p] = sum(w[k] << k for k in range(7))
    return perm


def _build_k2(weights_l):
    m = np.array([[1.0]], dtype=np.complex128)
    for w in (9, 8, 7, 6, 5, 4, 3):
        c, s = np.cos(weights_l[w] / 2), np.sin(weights_l[w] / 2)
        r = np.array([[c, -1j * s], [-1j * s, c]], dtype=np.complex128)
        m = np.kron(m, r)
    qa = _perm_matrix(_cnot_chain_perm_p())
    k2 = qa @ m
    k2b = k2 @ _perm_matrix(np.arange(128) ^ 1)
    return k2, k2b


def _make_shared(weights):
    """mats [128, 4*6*128] and the scal block, shared by all cores."""
    wt = weights.astype(np.float64).reshape(DEPTH, NQ)
    mats = np.zeros((128, DEPTH * 6 * 128), dtype=np.float32)
    scal = np.zeros((128, 32), dtype=np.float32)
    T = 1.0
    for l in range(DEPTH):
        k2, k2b = _build_k2(wt[l])
        blocks = [
            k2.real.T, (-k2.imag).T, k2.imag.T,
            k2b.real.T, (-k2b.imag).T, k2b.imag.T,
        ]
        for m_i, blk in enumerate(blocks):
            c0 = (l * 6 + m_i) * 128
            mats[:, c0:c0 + 128] = blk.astype(np.float32)
        for k, w in enumerate((0, 1, 2)):
            t = np.tan(wt[l, w] / 2)
            scal[:, l * 8 + 2 * k] = t
            scal[:, l * 8 + 2 * k + 1] = -t
            T *= np.cos(wt[l, w] / 2)
    scal[:, 31] = -0.5 * T * T
    return mats, scal


def _make_packed(features_core, scal):
    """Packed per-core const tensor [128, C_TOT]."""
    B = features_core.shape[0]
    th = features_core.astype(np.float64)
    c_emb, s_emb = np.cos(th / 2), np.sin(th / 2)
    v = np.stack([c_emb.astype(np.complex128), -1j * s_emb], axis=-1)

    # seed over wires 3..7: j = w7*16 + w6*8 + w5*4 + w4*2 + w3
    seed = np.empty((B, 32), dtype=np.complex128)
    for j in range(32):
        val = np.ones(B, dtype=np.complex128)
        for k, w in enumerate((3, 4, 5, 6, 7)):
            val = val * v[:, w, (j >> k) & 1]
        seed[:, j] = val

    F = np.empty((B, 8), dtype=np.complex128)
    for g in range(8):
        w0, w1, w2 = (g >> 2) & 1, (g >> 1) & 1, g & 1
        F[:, g] = v[:, 0, w0] * v[:, 1, w1] * v[:, 2, w2]
    # free col = bh*128 + g*16 + bl
    fbd = np.zeros((B, 8 * B), dtype=np.complex128)
    for b in range(B):
        bh, bl = divmod(b, HB)
        for g in range(8):
            fbd[b, bh * 128 + g * HB + bl] = F[b, g]

    packed = np.zeros((128, C_TOT), dtype=np.float32)
    packed[:, C_SCAL:C_SCAL + 32] = scal
    packed[0:B, C_SEED_RE:C_SEED_RE + 32] = seed.real
    packed[0:B, C_SEED_IM:C_SEED_IM + 32] = seed.imag
    packed[0:B, C_DBL_C] = c_emb[:, 8]
    packed[0:B, C_DBL_C + 1] = c_emb[:, 9]
    packed[0:B, C_DBL_S] = s_emb[:, 8]
    packed[0:B, C_DBL_S + 1] = s_emb[:, 9]
    packed[0:B, C_FIMN:C_FIMN + 256] = -fbd.imag
    packed[0:B, C_FRE:C_FRE + 256] = fbd.real
    packed[0:B, C_FIM:C_FIM + 256] = fbd.imag
    return packed


# ---------------------------------------------------------------------------
# Bass program
# ---------------------------------------------------------------------------

_PROGRAM = None


def _build_program(layer_reps=1):
    import concourse.bacc as bacc
    import concourse.mybir as mybir
    import concourse.tile as tile

    F32 = mybir.dt.float32
    MULT = mybir.AluOpType.mult
    ADD = mybir.AluOpType.add
    B = B_CORE

    nc = bacc.Bacc("TRN2", target_bir_lowering=False, debug=False,
                   num_devices=NCORES)

    d_pk = nc.dram_tensor("packed", [128, C_TOT], F32, kind="ExternalInput")
    d_mats = nc.dram_tensor("mats", [128, DEPTH * 6 * 128], F32,
                            kind="ExternalInput")
    d_out = nc.dram_tensor("out", [1, B], F32, kind="ExternalOutput")

    with tile.TileContext(nc) as tc:
        with (
            tc.tile_pool(name="const", bufs=1) as cpool,
            tc.tile_pool(name="state", bufs=10) as spool,
            tc.tile_pool(name="psum", bufs=6, space="PSUM") as ppool,
                                    tc.tile_pool(name="psumq", bufs=2, space="PSUM") as ppool_q,
        ):
            t_pk = cpool.tile([128, C_TOT], F32, tag="pk")
            t_mats = cpool.tile([128, DEPTH * 6 * 128], F32, tag="mats")
            t_ones = cpool.tile([16, 1], F32, tag="ones")
            t_wu = cpool.tile([128, 128], F32, tag="wu")

            # PE warm-up: junk matmuls to lift the clock gate while DMAs run
            nc.vector.memset(t_wu[:], 0.0)
            ps_wu = ppool_q.tile([128, 128], F32, tag="pq")
            for _ in range(7):
                nc.tensor.matmul(ps_wu[:], t_wu[:], t_wu[:],
                                 start=True, stop=True)

            nc.sync.dma_start(t_pk[0:B, 32:C_FIMN], d_pk[0:B, 32:C_FIMN])
            nc.sync.dma_start(t_pk[0:B, C_FIMN:], d_pk[0:B, C_FIMN:])
            nc.sync.dma_start(t_pk[:, 0:32], d_pk[:, 0:32])
            for l in range(DEPTH):
                c0 = l * 6 * 128
                nc.sync.dma_start(t_mats[:, c0:c0 + 768],
                                  d_mats[:, c0:c0 + 768])
            nc.vector.memset(t_ones[:], 1.0)

            def scal_ap(col, p=128):
                return t_pk[0:p, C_SCAL + col:C_SCAL + col + 1]

            # ---------------- embedding ----------------
            pt_re = spool.tile([B, 128], F32, tag="pt")
            pt_im = spool.tile([B, 128], F32, tag="pt")
            nc.vector.tensor_copy(pt_re[:, 0:32],
                                  t_pk[0:B, C_SEED_RE:C_SEED_RE + 32])
            nc.vector.tensor_copy(pt_im[:, 0:32],
                                  t_pk[0:B, C_SEED_IM:C_SEED_IM + 32])
            k = 32
            for j in range(2):  # wires 8, 9
                c_ap = t_pk[0:B, C_DBL_C + j:C_DBL_C + j + 1]
                s_ap = t_pk[0:B, C_DBL_S + j:C_DBL_S + j + 1]
                nc.vector.tensor_scalar(
                    pt_re[:, k:2 * k], pt_im[:, 0:k], s_ap, None, op0=MULT)
                nc.vector.tensor_scalar(
                    pt_im[:, k:2 * k], pt_re[:, 0:k], s_ap, -1.0,
                    op0=MULT, op1=MULT)
                nc.vector.tensor_scalar(
                    pt_re[:, 0:k], pt_re[:, 0:k], c_ap, None, op0=MULT)
                nc.vector.tensor_scalar(
                    pt_im[:, 0:k], pt_im[:, 0:k], c_ap, None, op0=MULT)
                k *= 2

            # S = PT.T @ Fbd, complex, stacked into one [128, 512] PSUM:
            #   psum = PTre @ [Fre | Fim] + PTim @ [Fimn | Fre] = [Sre | Sim]
            ps_s = ppool.tile([128, 512], F32, tag="ps")
            psv = ps_s[:].rearrange("p (i r) -> p i r", i=2, r=256)
            rhs1 = t_pk[0:B, C_FRE:C_FRE + 512].rearrange(
                "p (i r) -> p i r", i=2, r=256)
            rhs2 = t_pk[0:B, C_FIMN:C_FIMN + 512].rearrange(
                "p (i r) -> p i r", i=2, r=256)
            for hb in range(2):
                c0, c1 = hb * 128, hb * 128 + 128
                nc.tensor.matmul(psv[:, :, c0:c1], pt_re[:],
                                 rhs1[:, :, c0:c1], start=True, stop=False)
                nc.tensor.matmul(psv[:, :, c0:c1], pt_im[:],
                                 rhs2[:, :, c0:c1], start=False, stop=True)

            s_re = spool.tile([128, 8 * B], F32, tag="st")
            s_im = spool.tile([128, 8 * B], F32, tag="st")
            for hb in range(2):
                c0 = hb * 128
                nc.scalar.copy(s_re[:, c0:c0 + 128], ps_s[:, c0:c0 + 128])
                nc.scalar.copy(s_im[:, c0:c0 + 128],
                               ps_s[:, 256 + c0:256 + c0 + 128])

            # ---------------- entangler layers ----------------
            # per-half views (cols hb*128 .. hb*128+128): g-major, bl inner
            def half(t, hb, p0=0, p1=128):
                return t[p0:p1, hb * 128:hb * 128 + 128]

            def vi(t, hb):  # [p, 2 (w0), 64]
                return half(t, hb).rearrange("p (i r) -> p i r", i=2, r=64)

            def vu(t, hb, i):  # fixed w0 half -> [p, 2 (w1), 32]
                return half(t, hb).rearrange(
                    "p (i m r) -> p i m r", i=2, m=2, r=32)[:, i]

            def vq(t, hb, q):  # g-pair q -> [p, 2 (w2), 16]
                return half(t, hb).rearrange(
                    "p (q s b) -> p q s b", q=4, s=2, b=HB)[:, q]

            def vg(t, hb, p0=0, p1=128):  # [p, 8 (g), 16]
                return half(t, hb, p0, p1).rearrange(
                    "p (g b) -> p g b", g=8, b=HB)

            for rep in range(layer_reps):
              for l in range(DEPTH):
                is_last = rep == layer_reps - 1 and l == DEPTH - 1

                def tp(k):
                    return scal_ap(l * 8 + 2 * k)

                def tn(k):
                    return scal_ap(l * 8 + 2 * k + 1)

                a_re = spool.tile([128, 8 * B], F32, tag="st")
                a_im = spool.tile([128, 8 * B], F32, tag="st")
                b_re = spool.tile([128, 8 * B], F32, tag="st")
                b_im = spool.tile([128, 8 * B], F32, tag="st")
                c_re = spool.tile([128, 8 * B], F32, tag="st")
                c_im = spool.tile([128, 8 * B], F32, tag="st")
                pm_re = [None, None]
                pm_im = [None, None]

                for hb in range(2):
                    # R0: whole-half STT, w0 halves swapped on in0
                    nc.vector.scalar_tensor_tensor(
                        vi(a_re, hb), vi(s_im, hb)[:, ::-1, :], tp(0),
                        vi(s_re, hb), op0=MULT, op1=ADD)
                    nc.vector.scalar_tensor_tensor(
                        vi(a_im, hb), vi(s_re, hb)[:, ::-1, :], tn(0),
                        vi(s_im, hb), op0=MULT, op1=ADD)
                    # R1 per w0-half
                    for i in range(2):
                        nc.vector.scalar_tensor_tensor(
                            vu(b_re, hb, i), vu(a_im, hb, i)[:, ::-1, :],
                            tp(1), vu(a_re, hb, i), op0=MULT, op1=ADD)
                        nc.vector.scalar_tensor_tensor(
                            vu(b_im, hb, i), vu(a_re, hb, i)[:, ::-1, :],
                            tn(1), vu(a_im, hb, i), op0=MULT, op1=ADD)
                    # R2 + pi: out_q <- in1(b, maybe pair-swapped) + t2*in0
                    for (qo, qi, rev) in (
                        (0, 0, False), (1, 1, True), (2, 3, False),
                        (3, 2, True),
                    ):
                        for (dst, p1, p0, sc) in (
                            (c_re, b_re, b_im, tp(2)),
                            (c_im, b_im, b_re, tn(2)),
                        ):
                            if rev:
                                in1 = vq(p1, hb, qi)[:, ::-1, :]
                                in0 = vq(p0, hb, qi)
                            else:
                                in1 = vq(p1, hb, qi)
                                in0 = vq(p0, hb, qi)[:, ::-1, :]
                            nc.vector.scalar_tensor_tensor(
                                vq(dst, hb, qo), in0, sc, in1,
                                op0=MULT, op1=ADD)

                    # matmul: even g -> K2, odd g -> K2b
                    pm_re[hb] = ppool.tile([128, 128], F32, tag="ps", name=f"pmre{rep}_{l}{hb}")
                    pm_im[hb] = ppool.tile([128, 128], F32, tag="ps", name=f"pmim{rep}_{l}{hb}")

                    def mat(mi):
                        c0 = (l * 6 + mi) * 128
                        return t_mats[:, c0:c0 + 128]

                    pv_re = pm_re[hb][:].rearrange(
                        "p (g b) -> p g b", g=8, b=HB)
                    pv_im = pm_im[hb][:].rearrange(
                        "p (g b) -> p g b", g=8, b=HB)
                    for par, m0 in ((0, 0), (1, 3)):
                        xre = vg(c_re, hb)[:, par::2, :]
                        xim = vg(c_im, hb)[:, par::2, :]
                        nc.tensor.matmul(pv_re[:, par::2, :], mat(m0 + 0),
                                         xre, start=True, stop=False)
                        nc.tensor.matmul(pv_re[:, par::2, :], mat(m0 + 1),
                                         xim, start=False, stop=True)
                        nc.tensor.matmul(pv_im[:, par::2, :], mat(m0 + 2),
                                         xre, start=True, stop=False)
                        nc.tensor.matmul(pv_im[:, par::2, :], mat(m0 + 0),
                                         xim, start=False, stop=True)

                if not is_last:
                    s_re = spool.tile([128, 8 * B], F32, tag="st")
                    s_im = spool.tile([128, 8 * B], F32, tag="st")
                    for hb in range(2):
                        for (dst, src) in ((s_re, pm_re[hb]),
                                           (s_im, pm_im[hb])):
                            sv = src[:].rearrange("p (g b) -> p g b",
                                                  g=8, b=HB)
                            svh = src[:].rearrange("p (i r) -> p i r",
                                                   i=2, r=4 * HB)
                            # lower partitions: straight
                            nc.scalar.copy(vg(dst, hb, 0, 64), sv[0:64])
                            # upper: C90 fold (g ^= 4) = i-dim reversal
                            nc.scalar.copy(
                                half(dst, hb, 64, 128).rearrange(
                                    "p (i r) -> p i r", i=2, r=4 * HB),
                                svh[64:128, ::-1, :])
                else:
                    ss_re = [None, None]
                    ss_im = [None, None]
                    for hb in range(2):
                        ss_re[hb] = spool.tile([16, 128], F32, tag="fin", name=f"ssre{hb}")
                        ss_im[hb] = spool.tile([16, 128], F32, tag="fin", name=f"ssim{hb}")
                        nc.scalar.copy(ss_re[hb][:], pm_re[hb][0:16, :])
                        nc.scalar.copy(ss_im[hb][:], pm_im[hb][0:16, :])

            # ---------------- projection + output ----------------
            res = spool.tile([1, B], F32, tag="res")
            for hb in range(2):
                sq = spool.tile([16, 128], F32, tag="fin")
                sq2 = spool.tile([16, 128], F32, tag="fin")
                nc.vector.tensor_tensor(sq[:], ss_re[hb][:], ss_re[hb][:],
                                        op=MULT)
                nc.vector.tensor_tensor(sq2[:], ss_im[hb][:], ss_im[hb][:],
                                        op=MULT)
                nc.vector.tensor_tensor(sq[:], sq[:], sq2[:], op=ADD)
                q1 = spool.tile([16, HB], F32, tag="q1")
                nc.vector.tensor_reduce(
                    q1[:], sq[:].rearrange("p (g b) -> p b g", g=8, b=HB),
                    axis=mybir.AxisListType.X, op=ADD)
                pq = ppool_q.tile([1, HB], F32, tag="pq")
                nc.tensor.matmul(pq[:], t_ones[:], q1[:],
                                 start=True, stop=True)
                nc.vector.tensor_scalar(
                    res[:, hb * HB:hb * HB + HB], pq[:], scal_ap(31, 1),
                    0.5, op0=MULT, op1=ADD)
            nc.sync.dma_start(d_out[:], res[:])

    nc.compile()
    return nc


# ---------------------------------------------------------------------------
# Entry point
# ---------------------------------------------------------------------------


def kernel(features, weights):
    global _PROGRAM
    from concourse.bass_utils import run_bass_kernel_spmd

    features = np.asarray(features)
    weights = np.asarray(weights)
    if _PROGRAM is None:
        _PROGRAM = _build_program()
    nc = _PROGRAM

    mats, scal = _make_shared(weights)
    in_maps = []
    for c in range(NCORES):
        in_maps.append({
            "packed": _make_packed(
                features[c * B_CORE:(c + 1) * B_CORE], scal),
            "mats": mats,
        })

    # The NRT occasionally reports a transient "exec unit unrecoverable"
    # right after a prior process crashed; a fresh attempt succeeds.
    last_err = None
    for attempt in range(3):
        try:
            res = run_bass_kernel_spmd(nc, in_maps, list(range(NCORES)))
            break
        except Exception as e:  # noqa: BLE001
            last_err = e
            import time

            time.sleep(10 * (attempt + 1))
    else:
        raise last_err
    out = np.concatenate([res.results[c]["out"][0] for c in range(NCORES)])
    return out.astype(np.float32)


if __name__ == "__main__":
    rng = np.random.default_rng(0)
    f = rng.standard_normal((256, 10)).astype(np.float32)
    w = (0.01 * rng.random((4, 10))).astype(np.float32)
    print(kernel(f, w)[:8])



# revision 3
# speedup vs baseline: 2.9332x; 2.9332x over previous
"""Trainium2 Bass kernel v3: one-shot projected-circuit matmul.

q[b] = sum_{k<128} |sum_n M[k,n] psi0[n,b]|^2,  out = 0.5 - 0.5 q,
where M = P*U [128,1024] (P selects trash=000 rows) is host-precomputed
from the weights (shared across samples/cores), and psi0 is the per-sample
embedding product state.

Device: contraction over n in 8 chunks of 128 (n_lo = partitions).
Per chunk: 4 fp16 matmuls (32 cols each) with M-chunk as stationary,
accumulating out_re / out_im in PSUM [128, 32] across all 8 chunks.
Then |.|^2 (DVE), column-sum matmul with ones, affine, DMA out.

Blob layout [128, 2816] fp16, chunk-major so DMAs stream in compute order:
  chunk h cols [h*352, (h+1)*352): [psi_re 32 | psi_im 32 | -psi_im 32 |
                                    Mre^T 128 | Mim^T 128]
"""
import numpy as np

NCORES = 8
B_CORE = 32
DEPTH = 4
NQ = 10
N = 1 << NQ
CH = 352  # cols per chunk in blob

_PROGRAM = None


# ---------------------------------------------------------------------------
# Host-side: M = P*U via backward evolution of 128 selected bras
# ---------------------------------------------------------------------------

def _rx(t):
    c, s = np.cos(t / 2), np.sin(t / 2)
    return np.array([[c, -1j * s], [-1j * s, c]], dtype=np.complex128)


def _apply_1q(S, gate, wire):
    R = S.shape[0]
    a, b = 1 << wire, 1 << (NQ - wire - 1)
    S = S.reshape(R, a, 2, b)
    S = np.einsum("ij,rajc->raic", gate, S)
    return S.reshape(R, N)


def _apply_cnot(S, ctrl, tgt):
    n = np.arange(N)
    cbit = (n >> (NQ - 1 - ctrl)) & 1
    return S[:, n ^ (cbit << (NQ - 1 - tgt))]


def _build_M(weights):
    wts = weights.astype(np.float64).reshape(DEPTH, NQ)
    phi = np.zeros((128, N), dtype=np.complex128)
    phi[np.arange(128), np.arange(128) * 8] = 1.0
    for l in range(DEPTH - 1, -1, -1):
        for w in range(NQ - 1, -1, -1):
            phi = _apply_cnot(phi, w, (w + 1) % NQ)
        for w in range(NQ):
            phi = _apply_1q(phi, _rx(-wts[l, w]), w)
    return np.conj(phi)  # [128 rows k, 1024 cols n]


def _psi0(features):
    th = features.astype(np.float64)
    v = np.stack([np.cos(th / 2), -1j * np.sin(th / 2)], axis=-1)
    B = th.shape[0]
    S = np.ones((B, 1), dtype=np.complex128)
    for w in range(NQ):
        S = np.einsum("bi,bj->bij", S, v[:, w]).reshape(B, -1)
    return S  # [B, N]


def _make_blob(features_core, M):
    """[128, 2816] fp16 per core."""
    psi = _psi0(features_core)  # [32, 1024]
    blob = np.zeros((128, 8 * CH), dtype=np.float16)
    for h in range(8):
        ps = psi[:, h * 128:(h + 1) * 128].T  # [128 n_lo, 32 b]
        c0 = h * CH
        blob[:, c0:c0 + 32] = ps.real.astype(np.float16)
        blob[:, c0 + 32:c0 + 64] = ps.imag.astype(np.float16)
        blob[:, c0 + 64:c0 + 96] = (-ps.imag).astype(np.float16)
        Mc = M[:, h * 128:(h + 1) * 128]  # [128 k, 128 n_lo]
        blob[:, c0 + 96:c0 + 224] = Mc.real.T.astype(np.float16)
        blob[:, c0 + 224:c0 + 352] = Mc.imag.T.astype(np.float16)
    return blob


# ---------------------------------------------------------------------------
# Bass program
# ---------------------------------------------------------------------------

def _build_program():
    import concourse.bacc as bacc
    import concourse.mybir as mybir
    import concourse.tile as tile

    F16 = mybir.dt.float16
    F32 = mybir.dt.float32
    MULT = mybir.AluOpType.mult
    ADD = mybir.AluOpType.add

    nc = bacc.Bacc("TRN2", target_bir_lowering=False, debug=False,
                   num_devices=NCORES)

    d_blob = nc.dram_tensor("blob", [128, 8 * CH], F16, kind="ExternalInput")
    d_out = nc.dram_tensor("out", [1, B_CORE], F32, kind="ExternalOutput")

    with tile.TileContext(nc) as tc:
        with (
            tc.tile_pool(name="const", bufs=1) as cpool,
            tc.tile_pool(name="psum", bufs=1, space="PSUM") as ppool,
            tc.tile_pool(name="psq", bufs=1, space="PSUM") as qpool,
        ):
            t_blob = cpool.tile([128, 8 * CH], F16, tag="blob")
            t_ones = cpool.tile([128, 1], F16, tag="ones")
            t_sq = cpool.tile([128, 64], F16, tag="sq")
            t_res = cpool.tile([1, B_CORE], F32, tag="res")

            # stream chunks: 0 | 1-4 | 5-7
            nc.sync.dma_start(t_blob[:, 0:CH], d_blob[:, 0:CH])
            nc.sync.dma_start(t_blob[:, CH:5 * CH], d_blob[:, CH:5 * CH])
            nc.sync.dma_start(t_blob[:, 5 * CH:8 * CH],
                              d_blob[:, 5 * CH:8 * CH])
            nc.vector.memset(t_ones[:], 1.0)

            ps_re = ppool.tile([128, 32], F32, tag="re")
            ps_im = ppool.tile([128, 32], F32, tag="im")
            for h in range(8):
                c0 = h * CH
                pre = t_blob[:, c0:c0 + 32]
                pim = t_blob[:, c0 + 32:c0 + 64]
                pnim = t_blob[:, c0 + 64:c0 + 96]
                mre = t_blob[:, c0 + 96:c0 + 224]
                mim = t_blob[:, c0 + 224:c0 + 352]
                nc.tensor.matmul(ps_re[:], mre, pre,
                                 start=(h == 0), stop=False)
                nc.tensor.matmul(ps_re[:], mim, pnim,
                                 start=False, stop=(h == 7))
                nc.tensor.matmul(ps_im[:], mim, pre,
                                 start=(h == 0), stop=False)
                nc.tensor.matmul(ps_im[:], mre, pim,
                                 start=False, stop=(h == 7))

            SQ = mybir.ActivationFunctionType.Square
            nc.scalar.activation(t_sq[:, 0:32], ps_re[:], SQ)
            nc.scalar.activation(t_sq[:, 32:64], ps_im[:], SQ)
            psq = qpool.tile([1, 64], F32, tag="q")
            nc.tensor.matmul(psq[:], t_ones[:], t_sq[:],
                             start=True, stop=True)
            tq = cpool.tile([1, B_CORE], F32, tag="tq")
            nc.vector.tensor_reduce(
                tq[:], psq[:].rearrange("p (c b) -> p b c", c=2, b=32),
                axis=mybir.AxisListType.X, op=ADD)
            nc.vector.tensor_scalar(t_res[:], tq[:], -0.5, 0.5,
                                    op0=MULT, op1=ADD)
            nc.sync.dma_start(d_out[:], t_res[:])

    nc.compile()
    return nc


# ---------------------------------------------------------------------------
# Entry point
# ---------------------------------------------------------------------------

def kernel(features, weights):
    global _PROGRAM
    from concourse.bass_utils import run_bass_kernel_spmd

    features = np.asarray(features)
    weights = np.asarray(weights)
    if _PROGRAM is None:
        _PROGRAM = _build_program()
    nc = _PROGRAM

    M = _build_M(weights)
    in_maps = [{"blob": _make_blob(
        features[c * B_CORE:(c + 1) * B_CORE], M)} for c in range(NCORES)]

    last_err = None
    for attempt in range(3):
        try:
            res = run_bass_kernel_spmd(nc, in_maps, list(range(NCORES)))
            break
        except Exception as e:  # noqa: BLE001
            last_err = e
            import time

            time.sleep(10 * (attempt + 1))
    else:
        raise last_err
    out = np.concatenate([res.results[c]["out"][0] for c in range(NCORES)])
    return out.astype(np.float32)


if __name__ == "__main__":
    import jax
    jax.config.update("jax_platforms", "cpu")
    import reference
    from concourse.bass_interp import CoreSim
    from concourse.timeline_sim import TimelineSim

    inputs = {k: np.asarray(v) for k, v in reference.setup_inputs().items()}
    expected = np.asarray(reference.reference(**inputs))

    nc = _build_program()
    M = _build_M(inputs["weights"])
    sim = CoreSim(nc)
    sim.tensor("blob")[:] = _make_blob(inputs["features"][:B_CORE], M)
    sim.simulate()
    actual = np.asarray(sim.tensor("out")).ravel()
    exp = expected[:B_CORE]
    rel = np.abs(actual - exp) / np.maximum(np.abs(exp), 1e-12)
    print("expected[:5]:", exp[:5])
    print("actual[:5]:  ", actual[:5])
    print("CoreSim max rel err:", rel.max())
    print(f"TimelineSim: {TimelineSim(nc).simulate():.0f} ns")


# revision 11
# speedup vs baseline: 3.6114x; 1.2312x over previous
"""Trainium2 Bass kernel v5: one-shot projected-circuit matmul, packed blob.

q[b] = sum_{k<128} |sum_n M[k,n] psi0[n,b]|^2,  out = 0.5 - 0.5 q,
with M = P*U [128,1024] host-precomputed from the weights and psi0 the
per-sample embedding product state.

v5 over v3: M shipped as fp8 e5m2 (the big entries are ~1.0 where e5m2 is
near-exact; small entries are O(tan(w/2)) where 12.5% relative is far below
the 2e-2 gate), -psi_im built on device, blob packed as uint8 with bitcast
views, PE warmup matmuls to hold the high p-state.

Blob uint8 [128, 8*384], chunk h at h*384:
  [+0:+64)    psi_re fp16 (32)
  [+64:+128)  psi_im fp16 (32)
  [+128:+256) Mre^T  e5m2 (128)
  [+256:+384) Mim^T  e5m2 (128)
"""
import numpy as np

NCORES = 8
B_CORE = 32
DEPTH = 4
NQ = 10
N = 1 << NQ
CHB = 384  # bytes per chunk in blob

_PROGRAM = None


# ---------------------------------------------------------------------------
# Host-side: M = P*U via backward evolution of 128 selected bras
# ---------------------------------------------------------------------------

def _rx(t):
    c, s = np.cos(t / 2), np.sin(t / 2)
    return np.array([[c, -1j * s], [-1j * s, c]], dtype=np.complex128)


def _apply_1q(S, gate, wire):
    R = S.shape[0]
    a, b = 1 << wire, 1 << (NQ - wire - 1)
    S = S.reshape(R, a, 2, b)
    S = np.einsum("ij,rajc->raic", gate, S)
    return S.reshape(R, N)


def _apply_cnot(S, ctrl, tgt):
    n = np.arange(N)
    cbit = (n >> (NQ - 1 - ctrl)) & 1
    return S[:, n ^ (cbit << (NQ - 1 - tgt))]


def _build_M(weights):
    wts = weights.astype(np.float64).reshape(DEPTH, NQ)
    phi = np.zeros((128, N), dtype=np.complex128)
    phi[np.arange(128), np.arange(128) * 8] = 1.0
    for l in range(DEPTH - 1, -1, -1):
        for w in range(NQ - 1, -1, -1):
            phi = _apply_cnot(phi, w, (w + 1) % NQ)
        for w in range(NQ):
            phi = _apply_1q(phi, _rx(-wts[l, w]), w)
    return np.conj(phi)  # [128 rows k, 1024 cols n]


def _psi0(features):
    th = features.astype(np.float64)
    v = np.stack([np.cos(th / 2), -1j * np.sin(th / 2)], axis=-1)
    B = th.shape[0]
    S = np.ones((B, 1), dtype=np.complex128)
    for w in range(NQ):
        S = np.einsum("bi,bj->bij", S, v[:, w]).reshape(B, -1)
    return S  # [B, N]


def _make_blob(features_core, M):
    """[128, 8*CHB] uint8 per core."""
    import ml_dtypes

    E5 = ml_dtypes.float8_e5m2
    psi = _psi0(features_core)  # [32, 1024]
    blob = np.zeros((128, 8 * CHB), dtype=np.uint8)
    for h in range(8):
        ps = psi[:, h * 128:(h + 1) * 128].T  # [128 n_lo, 32 b]
        c0 = h * CHB
        blob[:, c0:c0 + 64] = np.ascontiguousarray(ps.real.astype(np.float16)).view(np.uint8)
        blob[:, c0 + 64:c0 + 128] = np.ascontiguousarray(ps.imag.astype(np.float16)).view(np.uint8)
        Mc = M[:, h * 128:(h + 1) * 128]  # [128 k, 128 n_lo]
        blob[:, c0 + 128:c0 + 256] = np.ascontiguousarray(Mc.real.T.astype(E5)).view(np.uint8)
        blob[:, c0 + 256:c0 + 384] = np.ascontiguousarray(Mc.imag.T.astype(E5)).view(np.uint8)
    return blob


# ---------------------------------------------------------------------------
# Bass program
# ---------------------------------------------------------------------------

def _build_program():
    import concourse.bacc as bacc
    import concourse.mybir as mybir
    import concourse.tile as tile

    F16 = mybir.dt.float16
    F32 = mybir.dt.float32
    F8 = mybir.dt.float8e5
    MULT = mybir.AluOpType.mult
    ADD = mybir.AluOpType.add

    nc = bacc.Bacc("TRN2", target_bir_lowering=False, debug=False,
                   num_devices=NCORES)

    d_blob = nc.dram_tensor("blob", [128, 8 * CHB], mybir.dt.uint8,
                            kind="ExternalInput")
    d_out = nc.dram_tensor("out", [1, B_CORE], F32, kind="ExternalOutput")

    with tile.TileContext(nc) as tc:
        with (
            tc.tile_pool(name="const", bufs=1) as cpool,
            tc.tile_pool(name="psum", bufs=1, space="PSUM") as ppool,
            tc.tile_pool(name="psj", bufs=1, space="PSUM") as jpool,
            tc.tile_pool(name="psq", bufs=1, space="PSUM") as qpool,
        ):
            t_blob = cpool.tile([128, 8 * CHB], mybir.dt.uint8, tag="blob")
            t_ones = cpool.tile([128, 1], F16, tag="ones")
            t_junk = cpool.tile([128, 512], F16, tag="junk")
            t_pnim = cpool.tile([128, 256], F16, tag="pnim")
            t_sq = cpool.tile([128, 64], F16, tag="sq")
            t_res = cpool.tile([B_CORE, 1], F32, tag="res")

            # stream chunks 0-3 | 4-7; only the last chunk gates the chain
            nc.sync.dma_start(t_blob[:, 0:5 * CHB], d_blob[:, 0:5 * CHB])
            nc.sync.dma_start(t_blob[:, 5 * CHB:8 * CHB],
                              d_blob[:, 5 * CHB:8 * CHB])
            nc.vector.memset(t_ones[:], 1.0)
            nc.vector.memset(t_junk[:], 0.0)

            # PE warmup: hold the tensor engine busy through the DMA wait so
            # the p-state is high when the real matmuls issue
            psj = jpool.tile([1, 512], F32, tag="j")
            for _ in range(3):
                nc.tensor.matmul(psj[:], t_ones[:], t_junk[:],
                                 start=True, stop=True)

            # -psi_im per DMA group (chunk-strided fp16 view of the blob)
            fview = t_blob[:].bitcast(F16).rearrange(
                "p (h c) -> p h c", h=8, c=CHB // 2)
            for g0, g1 in ((0, 5), (5, 8)):
                nc.vector.tensor_scalar(
                    t_pnim[:].rearrange("p (h c) -> p h c", h=8, c=32)
                    [:, g0:g1],
                    fview[:, g0:g1, 32:64], -1.0, None, op0=MULT)

            ps_re = ppool.tile([128, 32], F32, tag="re")
            ps_im = ppool.tile([128, 32], F32, tag="im")
            for h in range(8):
                c0 = h * CHB
                pre = t_blob[:, c0:c0 + 64].bitcast(F16)
                pim = t_blob[:, c0 + 64:c0 + 128].bitcast(F16)
                pnim = t_pnim[:, h * 32:h * 32 + 32]
                mre = t_blob[:, c0 + 128:c0 + 256].bitcast(F8)
                mim = t_blob[:, c0 + 256:c0 + 384].bitcast(F8)
                nc.tensor.matmul(ps_re[:], mre, pre,
                                 start=(h == 0), stop=False)
                nc.tensor.matmul(ps_re[:], mim, pnim,
                                 start=False, stop=(h == 7))
                nc.tensor.matmul(ps_im[:], mim, pre,
                                 start=(h == 0), stop=False)
                nc.tensor.matmul(ps_im[:], mre, pim,
                                 start=False, stop=(h == 7))

            SQ = mybir.ActivationFunctionType.Square
            nc.scalar.activation(t_sq[:, 0:32], ps_re[:], SQ)
            nc.scalar.activation(t_sq[:, 32:64], ps_im[:], SQ)
            # q[b] on partitions: contract the 128 i-partitions with ones,
            # folding re+im via two accumulating matmuls
            psq = qpool.tile([B_CORE, 1], F32, tag="q")
            nc.tensor.matmul(psq[:], t_sq[:, 0:32], t_ones[:],
                             start=True, stop=False)
            nc.tensor.matmul(psq[:], t_sq[:, 32:64], t_ones[:],
                             start=False, stop=True)
            nc.vector.tensor_scalar(t_res[:], psq[:], -0.5, 0.5,
                                    op0=MULT, op1=ADD)
            nc.sync.dma_start(d_out[:], t_res[:])

    nc.compile()
    return nc


# ---------------------------------------------------------------------------
# Entry point
# ---------------------------------------------------------------------------

def kernel(features, weights):
    global _PROGRAM
    from concourse.bass_utils import run_bass_kernel_spmd

    features = np.asarray(features)
    weights = np.asarray(weights)
    if _PROGRAM is None:
        _PROGRAM = _build_program()
    nc = _PROGRAM

    M = _build_M(weights)
    in_maps = [{"blob": _make_blob(
        features[c * B_CORE:(c + 1) * B_CORE], M)} for c in range(NCORES)]

    last_err = None
    for attempt in range(3):
        try:
            res = run_bass_kernel_spmd(nc, in_maps, list(range(NCORES)))
            break
        except Exception as e:  # noqa: BLE001
            last_err = e
            import time

            time.sleep(10 * (attempt + 1))
    else:
        raise last_err
    out = np.concatenate(
        [res.results[c]["out"].reshape(-1) for c in range(NCORES)])
    return out.astype(np.float32)


if __name__ == "__main__":
    import jax
    jax.config.update("jax_platforms", "cpu")
    import reference
    from concourse.bass_interp import CoreSim
    from concourse.timeline_sim import TimelineSim

    inputs = {k: np.asarray(v) for k, v in reference.setup_inputs().items()}
    expected = np.asarray(reference.reference(**inputs))

    nc = _build_program()
    M = _build_M(inputs["weights"])
    sim = CoreSim(nc)
    sim.tensor("blob")[:] = _make_blob(inputs["features"][:B_CORE], M)
    sim.simulate()
    actual = np.asarray(sim.tensor("out")).ravel()[:B_CORE]
    exp = expected[:B_CORE]
    rel = np.abs(actual - exp) / np.maximum(np.abs(exp), 1e-12)
    print("expected[:5]:", exp[:5])
    print("actual[:5]:  ", actual[:5])
    print("CoreSim max rel err:", rel.max())
    print(f"TimelineSim: {TimelineSim(nc).simulate():.0f} ns")
